# revision 1
# baseline (speedup 1.0000x reference)
"""Trainium2 Bass kernel for nn_CrossAttention_31791347925417.

Math (per batch b, per stream tok in {x, blood} with weight W in {W1, W2}):
    kv = tok @ W.T ; k, v heads [H, N, D]
    ctx = softmax_d( SCALE * k_h^T v_h )          # [H, D, D], softmax over first D
    out_x = x_h @ ctx2_h ; out_b = blood_h @ ctx1_h

Refactor used here (Gram trick):
    k_h^T v_h = W_k_h (tok^T tok) W_v_h^T  with G = tok^T tok  [C, C]
so the N=4096 contraction happens once (G) instead of twice (k and v), and
everything downstream is tiny [C,C]-scale work.

We compute ctxT_h = (SCALE*W_k applied) via  Q = G @ WkT, then per head-pair
a full [128,128] product  WvT_pair^T @ Q_pair  whose DIAGONAL 64x64 blocks are
ctxT_h [e, d] for the two heads (off-diagonal blocks are unused).  Softmax runs
along the free axis (d).  The normalized probs are written into the diagonal
blocks of a zeroed [128,128] tile F; BD = F^T (PE transpose) is the
block-diagonal ctx pair used by the output matmuls:
    out[n, (h,e)] = sum_{(h,d)} xT[(h,d), n] * BD[(h,d), (h,e)]

Sharding: data-parallel over batch B=8 across the 8 cores; weights replicated.
Host pre-transposes W -> W.T [C, 2C] and folds SCALE into the k-half (exact,
SCALE = 0.125).
"""

import sys

if "/opt/trn_rl_repo" not in sys.path:
    sys.path.insert(0, "/opt/trn_rl_repo")

import numpy as np

from concourse import bacc, masks, mybir, tile
from concourse.bass_utils import run_bass_kernel_spmd

B, N, C, H = 8, 4096, 512, 8
D = C // H
SCALE = D ** -0.5
P = 128
NBIG = N // 512          # 8 big row tiles (512 rows each)
NT = N // P              # 32 n-tiles
CB = C // P              # 4 column blocks == head pairs
F32 = mybir.dt.float32
F32R = mybir.dt.float32r
BF16 = mybir.dt.bfloat16
AX = mybir.AxisListType
ACT_EXP = mybir.ActivationFunctionType.Exp

# precision knobs
G_F32R = True      # G = tok^T tok in float32r (4x faster than float32)
Q_F32R = True      # Q = G @ WkT in float32r
OUT_BF16 = True    # final out matmuls in bf16 (vs float32)
TRANS_BF16_ID = False  # walrus rejects mixed f32r/bf16 matmul operands
TRANS_F32R = False  # walrus codegen rejects f32r transpose-mode


def _r(ap):
    return ap.bitcast(F32R)


def build_nc():
    nc = bacc.Bacc("TRN2", target_bir_lowering=False, debug=False)

    TOKDT = F32R if G_F32R else F32
    WDT = F32R if Q_F32R else F32
    xb = nc.dram_tensor("xb", [N, C], TOKDT, kind="ExternalInput").ap()
    bb = nc.dram_tensor("bb", [N, C], TOKDT, kind="ExternalInput").ap()
    w1t = nc.dram_tensor("w1t", [C, 2 * C], WDT, kind="ExternalInput").ap()
    w2t = nc.dram_tensor("w2t", [C, 2 * C], WDT, kind="ExternalInput").ap()
    # blocked transposed output layout: [kb, part(c within pair), pair, n-col]
    ox = nc.dram_tensor("oxT", [NBIG, P, CB, 512], F32, kind="ExternalOutput").ap()
    ob = nc.dram_tensor("obT", [NBIG, P, CB, 512], F32, kind="ExternalOutput").ap()

    with tile.TileContext(nc) as tc:
        _emit(nc, tc, xb, bb, w1t, w2t, ox, ob)

    nc.compile()
    return nc


def _emit(nc, tc, xb, bb, w1t, w2t, ox, ob):
    TOKDT = F32R if G_F32R else F32
    WDT = F32R if Q_F32R else F32
    from contextlib import ExitStack

    ctx = ExitStack()
    with ctx:
        const = ctx.enter_context(tc.tile_pool(name="const", bufs=1))
        wpool = ctx.enter_context(tc.tile_pool(name="wpool", bufs=1))
        tokp = ctx.enter_context(tc.tile_pool(name="tokp", bufs=8))
        xtp = ctx.enter_context(tc.tile_pool(name="xtp", bufs=1))
        gqp = ctx.enter_context(tc.tile_pool(name="gqp", bufs=8))
        smallp = ctx.enter_context(tc.tile_pool(name="smallp", bufs=2))
        fpool = ctx.enter_context(tc.tile_pool(name="fpool", bufs=2))
        bdpool = ctx.enter_context(tc.tile_pool(name="bdpool", bufs=8))
        ostp = ctx.enter_context(tc.tile_pool(name="ostp", bufs=3))
        psG = ctx.enter_context(tc.tile_pool(name="psG", bufs=4, space="PSUM"))
        psT = ctx.enter_context(tc.tile_pool(name="psT", bufs=2, space="PSUM"))
        psO = ctx.enter_context(tc.tile_pool(name="psO", bufs=2, space="PSUM"))

        ident = const.tile([P, P], F32, tag="idf")
        masks.make_identity(nc, ident[:])
        ident_bf = const.tile([P, P], BF16, tag="idb")
        masks.make_identity(nc, ident_bf[:])
        if TRANS_F32R:
            ident_r = const.tile([P, P], F32R, tag="idr")
            masks.make_identity(nc, ident_r[:])
        else:
            ident_r = None

        # weights: chunk j (c-rows 128j..128j+128) lives at cols [j*2C, (j+1)*2C)
        w_x = wpool.tile([P, CB * 2 * C], WDT, tag="wx")
        w_b = wpool.tile([P, CB * 2 * C], WDT, tag="wb")

        def load_weights():
            nc.sync.dma_start(
                w_x[:].rearrange("p (j c) -> p j c", j=CB),
                w1t[:, :].rearrange("(j p) c -> p j c", p=P),
            )
            nc.sync.dma_start(
                w_b[:].rearrange("p (j c) -> p j c", j=CB),
                w2t[:, :].rearrange("(j p) c -> p j c", p=P),
            )

        def wchunk(w, j):
            return w[:, j * 2 * C:(j + 1) * 2 * C]

        # transposed tokens, bf16: pair block m at cols [m*N, (m+1)*N)
        xT_x = xtp.tile([P, CB * N], BF16, tag="xtx")
        xT_b = xtp.tile([P, CB * N], BF16, tag="xtb")

        def emit_loads(tok_dram, split_first=False):
            toks = []
            for kb in range(NBIG):
                tokb = tokp.tile([P, 4 * C], TOKDT, tag="tok", name=f"tok{kb}")
                if kb == 0 and split_first:
                    for sub in range(4):
                        nc.sync.dma_start(
                            tokb[:, sub * C:(sub + 1) * C],
                            tok_dram[kb * 512 + sub * P:kb * 512 + (sub + 1) * P, :],
                        )
                else:
                    nc.sync.dma_start(
                        tokb[:].rearrange("p (s c) -> p s c", s=4),
                        tok_dram[kb * 512:(kb + 1) * 512, :].rearrange(
                            "(s p) c -> p s c", p=P
                        ),
                    )
                toks.append(tokb)
            return toks

        # G is symmetric: row-block m only needs columns >= G_OFF[m]
        # (row 3 starts at 256 to keep the f32r moving dim >= 256).
        G_OFF = [0, P, 2 * P, 2 * P]

        def emit_G_tile(gps, sb, k):
            for m in range(CB):
                o = G_OFF[m]
                nc.tensor.matmul(
                    gps[m][:, o:C], sb[:, m * P:(m + 1) * P], sb[:, o:C],
                    start=(k == 0), stop=(k == NT - 1),
                )

        def emit_T_tile(xT, sb, k, alt):
            tpool, ttag = (psT, "t") if (not alt or k % 2 == 0) else (psO, "o")
            tps = tpool.tile([P, C], F32, tag=ttag, name="tps")
            for m in range(CB):
                nc.tensor.transpose(
                    tps[:, m * P:(m + 1) * P],
                    sb[:, m * P:(m + 1) * P].bitcast(F32), ident[:],
                )
            nc.vector.tensor_copy(
                xT[:].rearrange("p (m n) -> p m n", m=CB)[:, :, k * P:(k + 1) * P],
                tps[:].rearrange("p (m n) -> p m n", m=CB),
            )

        def out_chunk(xT, BDs, kb, ost, pool, ptag):
            """outT for 512 n-cols (tile-group kb): per pair p one matmul
            [c-block p, 512 n]; drain into ost quarter p (split DVE/ACT)."""
            for p in range(CB):
                ops = pool.tile([P, 512], F32, tag=ptag, name=f"ops{p}")
                nc.tensor.matmul(
                    ops[:], BDs[p][:], xT[:, p * N + kb * 512:p * N + (kb + 1) * 512],
                    start=True, stop=True,
                )
                if p % 2:
                    nc.scalar.copy(ost[:, p * 512:(p + 1) * 512], ops[:])
                else:
                    nc.vector.tensor_copy(ost[:, p * 512:(p + 1) * 512], ops[:])

        def emit_chain(gps, w):
            """G psum -> Q -> per-pair ctxT blocks -> softmax -> BD tiles.
            Q/ctx psums live in the "o" pool so the next stream's G can claim
            the "g" banks immediately."""
            g_sb = []
            for m in range(CB):
                o = G_OFF[m]
                g = gqp.tile([P, C], WDT, tag="gq", name=f"g{m}")
                nc.scalar.copy(g[:, o:C], gps[m][:, o:C])
                g_sb.append(g)
            # mirror missing lower blocks (i,j), j < G_OFF[i]//P, from (j,i)^T
            for i in range(CB):
                for j in range(G_OFF[i] // P):
                    mps = psT.tile([P, P], F32, tag="t", name="mps")
                    nc.tensor.transpose(
                        mps[:], g_sb[j][:, i * P:(i + 1) * P].bitcast(F32),
                        ident[:],
                    )
                    nc.scalar.copy(g_sb[i][:, j * P:(j + 1) * P], mps[:])

            q_sb = [None] * CB
            for i in reversed(range(CB)):
                qp = psO.tile([P, C], F32, tag="o", name=f"qp{i}")
                for j in range(CB):
                    nc.tensor.matmul(
                        qp[:], g_sb[j][:, i * P:(i + 1) * P],
                        wchunk(w, j)[:, 0:C], start=(j == 0), stop=(j == 3),
                    )
                q = gqp.tile([P, C], WDT, tag="gq", name=f"q{i}")
                nc.scalar.copy(q[:], qp[:])
                q_sb[i] = q

            BDs = []
            for p in range(CB):
                cps = psO.tile([P, P], F32, tag="o", name=f"cps{p}")
                for j in range(CB):
                    nc.tensor.matmul(
                        cps[:],
                        wchunk(w, j)[:, C + p * P:C + (p + 1) * P],
                        q_sb[j][:, p * P:(p + 1) * P],
                        start=(j == 0), stop=(j == 3),
                    )
                nm = smallp.tile([P, 1], F32, tag="nm", name="nm")
                sm = smallp.tile([P, 1], F32, tag="sm", name="sm")
                rv = smallp.tile([P, 1], F32, tag="rv", name="rv")
                pp = smallp.tile([P, D], F32, tag="pp", name="pp")
                fp = fpool.tile([P, P], BF16, tag="F", name="fp")
                nc.gpsimd.memset(fp[:], 0.0)
                for dd in range(2):
                    s0 = slice(dd * D, (dd + 1) * D)
                    blk = cps[s0, s0]
                    nc.vector.reduce_max(nm[s0, :], blk, axis=AX.X, negate=True)
                    nc.scalar.activation(
                        pp[s0, :], blk, ACT_EXP, bias=nm[s0, :], scale=1.0,
                        accum_out=sm[s0, :],
                    )
                nc.vector.reciprocal(rv[:], sm[:])
                for dd in range(2):
                    s0 = slice(dd * D, (dd + 1) * D)
                    nc.vector.tensor_scalar_mul(fp[s0, s0], pp[s0, :], rv[s0, :])
                bps = psT.tile([P, P], BF16, tag="t", name="bps")
                nc.tensor.transpose(bps[:, 0:P], fp[:], ident_bf[:])
                bd = bdpool.tile([P, P], BF16, tag="bd", name=f"bd{p}")
                nc.vector.tensor_copy(bd[:], bps[:, 0:P])
                BDs.append(bd)
            return BDs

        # ---- schedule ----
        toks_x = emit_loads(xb, split_first=True)
        load_weights()
        toks_b = emit_loads(bb)

        # phase A: interleaved G_x + T_x per tile (DMA-bound window)
        gps_x = [psG.tile([P, C], F32, tag="g", name=f"gpsx{m}") for m in range(CB)]
        for kb in range(NBIG):
            for sub in range(4):
                k = kb * 4 + sub
                sb = toks_x[kb][:, sub * C:(sub + 1) * C]
                emit_G_tile(gps_x, sb, k)
                emit_T_tile(xT_x, sb, k, alt=True)
        bd1 = emit_chain(gps_x, w_x)

        # B1: dense G_b (claims the "g" banks as soon as chain A drains them)
        gps_b = [psG.tile([P, C], F32, tag="g", name=f"gpsb{m}") for m in range(CB)]
        for kb in range(NBIG):
            for sub in range(4):
                k = kb * 4 + sub
                emit_G_tile(gps_b, toks_b[kb][:, sub * C:(sub + 1) * C], k)
        bd2 = emit_chain(gps_b, w_b)

        # B2: production loop — transpose blood, then both outputs per kb;
        # writes stream at full DMA rate from here on
        for kb in range(NBIG):
            for sub in range(4):
                k = kb * 4 + sub
                emit_T_tile(xT_b, toks_b[kb][:, sub * C:(sub + 1) * C], k, alt=False)
            ost_b = ostp.tile([P, 4 * 512], F32, tag="ost", name="ost_b")
            out_chunk(xT_b, bd1, kb, ost_b, psO, "o")
            nc.scalar.dma_start(ob[kb], ost_b[:])
            ost_x = ostp.tile([P, 4 * 512], F32, tag="ost", name="ost_x")
            out_chunk(xT_x, bd2, kb, ost_x, psG, "g")
            nc.scalar.dma_start(ox[kb], ost_x[:])


_NC_CACHE = None


def _get_nc():
    global _NC_CACHE
    if _NC_CACHE is None:
        _NC_CACHE = build_nc()
    return _NC_CACHE


def _prep_inputs(x, blood, W1, W2):
    x = np.ascontiguousarray(np.asarray(x, dtype=np.float32))
    blood = np.ascontiguousarray(np.asarray(blood, dtype=np.float32))
    w1t = np.ascontiguousarray(np.asarray(W1, dtype=np.float32).T)
    w2t = np.ascontiguousarray(np.asarray(W2, dtype=np.float32).T)
    w1t[:, :C] *= SCALE  # fold softmax scale into the k-projection (exact: 2^-3)
    w2t[:, :C] *= SCALE
    return [
        {"xb": x[b], "bb": blood[b], "w1t": w1t, "w2t": w2t} for b in range(B)
    ]


def _unshuffle(arr):
    """[NBIG, P, CB, 512] blocked-transposed -> [N, C] natural."""
    # arr[kb, part, p, col] = out[kb*512 + col, p*128 + part]
    return np.ascontiguousarray(
        arr.transpose(0, 3, 2, 1).reshape(N, C))


def kernel(x, blood, W1, W2, trace=False):
    nc = _get_nc()
    in_maps = _prep_inputs(x, blood, W1, W2)
    res = run_bass_kernel_spmd(nc, in_maps, core_ids=list(range(B)), trace=trace)
    out_x = np.stack([_unshuffle(res.results[b]["oxT"]) for b in range(B)])
    out_b = np.stack([_unshuffle(res.results[b]["obT"]) for b in range(B)])
    if trace:
        kernel.last_results = res
    return (out_x, out_b)



# revision 4
# speedup vs baseline: 1.3153x; 1.3153x over previous
"""Trainium2 Bass kernel for nn_CrossAttention_31791347925417.

Math (per batch b, per stream tok in {x, blood} with weight W in {W1, W2}):
    kv = tok @ W.T ; k, v heads [H, N, D]
    ctx = softmax_d( SCALE * k_h^T v_h )          # [H, D, D], softmax over first D
    out_x = x_h @ ctx2_h ; out_b = blood_h @ ctx1_h

Gram trick: k_h^T v_h = W_k_h (tok^T tok) W_v_h^T with G = tok^T tok [C, C], so
the N=4096 contraction happens once per stream; everything downstream is tiny
[C,C]-scale work.  ctx probs are written into block-diagonal BD tiles used by
the output matmuls out[n, (h,e)] = sum_{(h,d)} tokT[(h,d), n] * BD[(h,d), (h,e)].

This version is fully fp16 on-chip (validated ~2e-3 rel err vs the 2e-2 gate):
  - Host supplies tokens BOTH natural [N, C] (for G) and pre-transposed [C, N]
    (for the output matmuls) in fp16, so there are no on-chip token transposes
    and no PSUM->SBUF cast traffic for them.
  - All matmuls run at 1 cycle/row (fp16); G uses the true upper triangle
    (G_OFF = [0,128,256,384]) and mirrors the 6 lower blocks via PE transposes.
  - Outputs are written fp16 and upcast on the host.

Sharding: data-parallel over batch B=8 across the 8 cores; weights replicated.
Host pre-transposes W -> W.T [C, 2C], folds SCALE into the k-half (exact 2^-3),
and casts to fp16.
"""

import sys

if "/opt/trn_rl_repo" not in sys.path:
    sys.path.insert(0, "/opt/trn_rl_repo")

import numpy as np

from concourse import bacc, masks, mybir, tile
from concourse.bass_utils import run_bass_kernel_spmd

B, N, C, H = 8, 4096, 512, 8
D = C // H
SCALE = D ** -0.5
P = 128
NBIG = N // 512          # 8 big row tiles (512 rows each)
NT = N // P              # 32 n-tiles
CB = C // P              # 4 column blocks == head pairs
F32 = mybir.dt.float32
F16 = mybir.dt.float16
AX = mybir.AxisListType
ACT_EXP = mybir.ActivationFunctionType.Exp

# G row-block m computes columns [G_OFF[m], C) -- true upper triangle.
G_OFF = [0, P, 2 * P, 3 * P]


def build_nc():
    nc = bacc.Bacc("TRN2", target_bir_lowering=False, debug=False)

    xb = nc.dram_tensor("xb", [N, C], F16, kind="ExternalInput").ap()
    bb = nc.dram_tensor("bb", [N, C], F16, kind="ExternalInput").ap()
    xbt = nc.dram_tensor("xbt", [C, N], F16, kind="ExternalInput").ap()
    bbt = nc.dram_tensor("bbt", [C, N], F16, kind="ExternalInput").ap()
    w1t = nc.dram_tensor("w1t", [C, 2 * C], F16, kind="ExternalInput").ap()
    w2t = nc.dram_tensor("w2t", [C, 2 * C], F16, kind="ExternalInput").ap()
    # blocked transposed output layout: [kb, part(c within pair), pair, n-col]
    ox = nc.dram_tensor("oxT", [NBIG, P, CB, 512], F16, kind="ExternalOutput").ap()
    ob = nc.dram_tensor("obT", [NBIG, P, CB, 512], F16, kind="ExternalOutput").ap()

    with tile.TileContext(nc) as tc:
        _emit(nc, tc, xb, bb, xbt, bbt, w1t, w2t, ox, ob)

    nc.compile()
    return nc


def _emit(nc, tc, xb, bb, xbt, bbt, w1t, w2t, ox, ob):
    from contextlib import ExitStack

    ctx = ExitStack()
    with ctx:
        const = ctx.enter_context(tc.tile_pool(name="const", bufs=1))
        wpool = ctx.enter_context(tc.tile_pool(name="wpool", bufs=1))
        tokp = ctx.enter_context(tc.tile_pool(name="tokp", bufs=16))
        xtp = ctx.enter_context(tc.tile_pool(name="xtp", bufs=1))
        gqp = ctx.enter_context(tc.tile_pool(name="gqp", bufs=16))
        smallp = ctx.enter_context(tc.tile_pool(name="smallp", bufs=2))
        fpool = ctx.enter_context(tc.tile_pool(name="fpool", bufs=2))
        bdpool = ctx.enter_context(tc.tile_pool(name="bdpool", bufs=8))
        ostp = ctx.enter_context(tc.tile_pool(name="ostp", bufs=4))
        psA = ctx.enter_context(tc.tile_pool(name="psA", bufs=4, space="PSUM"))
        psB = ctx.enter_context(tc.tile_pool(name="psB", bufs=4, space="PSUM"))

        ident = const.tile([P, P], F16, tag="idh")
        masks.make_identity(nc, ident[:])

        # weights: chunk j (c-rows 128j..128j+128) lives at cols [j*2C, (j+1)*2C)
        w_x = wpool.tile([P, CB * 2 * C], F16, tag="wx")
        w_b = wpool.tile([P, CB * 2 * C], F16, tag="wb")

        def load_weights():
            nc.sync.dma_start(
                w_x[:].rearrange("p (j c) -> p j c", j=CB),
                w1t[:, :].rearrange("(j p) c -> p j c", p=P),
            )
            nc.sync.dma_start(
                w_b[:].rearrange("p (j c) -> p j c", j=CB),
                w2t[:, :].rearrange("(j p) c -> p j c", p=P),
            )

        def wchunk(w, j):
            return w[:, j * 2 * C:(j + 1) * 2 * C]

        # transposed tokens (from host): pair block m at cols [m*N, (m+1)*N)
        xT_x = xtp.tile([P, CB * N], F16, tag="xtx")
        xT_b = xtp.tile([P, CB * N], F16, tag="xtb")

        def load_xT(xT, tdram):
            for m in range(CB):
                nc.sync.dma_start(
                    xT[:, m * N:(m + 1) * N], tdram[m * P:(m + 1) * P, :]
                )

        def emit_loads(tok_dram, pfx, split_first=False):
            toks = []
            for kb in range(NBIG):
                tokb = tokp.tile([P, 4 * C], F16, tag="tok", name=f"{pfx}{kb}")
                if kb == 0 and split_first:
                    for sub in range(4):
                        nc.sync.dma_start(
                            tokb[:, sub * C:(sub + 1) * C],
                            tok_dram[kb * 512 + sub * P:kb * 512 + (sub + 1) * P, :],
                        )
                else:
                    nc.sync.dma_start(
                        tokb[:].rearrange("p (s c) -> p s c", s=4),
                        tok_dram[kb * 512:(kb + 1) * 512, :].rearrange(
                            "(s p) c -> p s c", p=P
                        ),
                    )
                toks.append(tokb)
            return toks

        def emit_G_tile(gps, sb, k):
            for m in range(CB):
                o = G_OFF[m]
                nc.tensor.matmul(
                    gps[m][:, o:C], sb[:, m * P:(m + 1) * P], sb[:, o:C],
                    start=(k == 0), stop=(k == NT - 1),
                )

        def drain_G(gps, eng):
            """PSUM G (f32) -> SBUF fp16 row-block tiles."""
            g_sb = []
            for m in range(CB):
                o = G_OFF[m]
                g = gqp.tile([P, C], F16, tag="gq", name=f"g{m}")
                eng(g[:, o:C], gps[m][:, o:C])
                g_sb.append(g)
            return g_sb

        def emit_mirrors(g_sb, pspool):
            """Fill missing lower blocks (i,j), j < G_OFF[i]//P, from (j,i)^T."""
            for i in range(CB):
                for j in range(G_OFF[i] // P):
                    mps = pspool.tile([P, P], F16, tag="g", name="mps")
                    nc.tensor.transpose(
                        mps[:], g_sb[j][:, i * P:(i + 1) * P], ident[:]
                    )
                    nc.vector.tensor_copy(g_sb[i][:, j * P:(j + 1) * P], mps[:])

        def emit_Q_ctx(g_sb, w, pspool):
            """Q = G @ Wk, then per-pair ctxT psum blocks (f32, kept in PSUM)."""
            q_sb = [None] * CB
            for i in reversed(range(CB)):
                qp = pspool.tile([P, C], F32, tag="g", name=f"qp{i}")
                for j in range(CB):
                    nc.tensor.matmul(
                        qp[:], g_sb[j][:, i * P:(i + 1) * P],
                        wchunk(w, j)[:, 0:C], start=(j == 0), stop=(j == 3),
                    )
                q = gqp.tile([P, C], F16, tag="gq", name=f"q{i}")
                nc.scalar.copy(q[:], qp[:])
                q_sb[i] = q
            cps_l = []
            for p in range(CB):
                cps = pspool.tile([P, P], F32, tag="g", name=f"cps{p}")
                for j in range(CB):
                    nc.tensor.matmul(
                        cps[:],
                        wchunk(w, j)[:, C + p * P:C + (p + 1) * P],
                        q_sb[j][:, p * P:(p + 1) * P],
                        start=(j == 0), stop=(j == 3),
                    )
                cps_l.append(cps)
            return cps_l

        def emit_softmax(cps_l):
            """softmax over d (free axis) of each diagonal 64x64 block; returns
            fp16 F tiles (block-diag probs, not yet transposed)."""
            fps = []
            for p in range(CB):
                cps = cps_l[p]
                nm = smallp.tile([P, 1], F32, tag="nm", name="nm")
                sm = smallp.tile([P, 1], F32, tag="sm", name="sm")
                rv = smallp.tile([P, 1], F32, tag="rv", name="rv")
                pp = smallp.tile([P, D], F32, tag="pp", name="pp")
                fp = fpool.tile([P, P], F16, tag="F", name="fp")
                nc.gpsimd.memset(fp[:], 0.0)
                for dd in range(2):
                    s0 = slice(dd * D, (dd + 1) * D)
                    blk = cps[s0, s0]
                    nc.vector.reduce_max(nm[s0, :], blk, axis=AX.X, negate=True)
                    nc.scalar.activation(
                        pp[s0, :], blk, ACT_EXP, bias=nm[s0, :], scale=1.0,
                        accum_out=sm[s0, :],
                    )
                nc.vector.reciprocal(rv[:], sm[:])
                for dd in range(2):
                    s0 = slice(dd * D, (dd + 1) * D)
                    nc.vector.tensor_scalar_mul(fp[s0, s0], pp[s0, :], rv[s0, :])
                fps.append(fp)
            return fps

        def emit_BD(fps, pspool):
            """PE-transpose prob tiles into block-diagonal ctx operands."""
            BDs = []
            for p in range(CB):
                bps = pspool.tile([P, P], F16, tag="g", name="bps")
                nc.tensor.transpose(bps[:], fps[p][:], ident[:])
                bd = bdpool.tile([P, P], F16, tag="bd", name=f"bd{p}")
                nc.vector.tensor_copy(bd[:], bps[:])
                BDs.append(bd)
            return BDs

        DRAIN = [nc.scalar.copy, nc.vector.tensor_copy]

        def out_chunk(xT, BDs, kb, odram, pool):
            """outT for 512 n-cols (tile-group kb): per pair p one matmul
            [c-block p, 512 n]; drain into ost quarter p rotating engines."""
            ost = ostp.tile([P, 4 * 512], F16, tag="ost", name="ost")
            for p in range(CB):
                ops = pool.tile([P, 512], F32, tag="g", name=f"ops{p}")
                nc.tensor.matmul(
                    ops[:], BDs[p][:], xT[:, p * N + kb * 512:p * N + (kb + 1) * 512],
                    start=True, stop=True,
                )
                DRAIN[p % 2](ost[:, p * 512:(p + 1) * 512], ops[:])
            nc.scalar.dma_start(odram[kb], ost[:])

        # ---- emission schedule ----
        toks_x = emit_loads(xb, "tx", split_first=True)
        load_weights()
        toks_b = emit_loads(bb, "tb")
        load_xT(xT_b, bbt)
        load_xT(xT_x, xbt)

        # G_x: 32 k-tiles, 4 psum banks
        gps_x = [psA.tile([P, C], F32, tag="g", name=f"gpsx{m}") for m in range(CB)]
        for kb in range(NBIG):
            for sub in range(4):
                emit_G_tile(gps_x, toks_x[kb][:, sub * C:(sub + 1) * C], kb * 4 + sub)
        g_sb_x = drain_G(gps_x, nc.scalar.copy)       # scalar, runs during G_b

        # G_b first half
        gps_b = [psB.tile([P, C], F32, tag="g", name=f"gpsb{m}") for m in range(CB)]
        for kb in range(NBIG // 2):
            for sub in range(4):
                emit_G_tile(gps_b, toks_b[kb][:, sub * C:(sub + 1) * C], kb * 4 + sub)

        # mirrors for x slot in mid-G_b (their psum comes from freed psA banks)
        emit_mirrors(g_sb_x, psA)

        # G_b second half
        for kb in range(NBIG // 2, NBIG):
            for sub in range(4):
                emit_G_tile(gps_b, toks_b[kb][:, sub * C:(sub + 1) * C], kb * 4 + sub)
        g_sb_b = drain_G(gps_b, nc.vector.tensor_copy)  # DVE, frees scalar

        # chain x compute: Q, ctx (PE) then softmax (DVE/ACT)
        cps_x = emit_Q_ctx(g_sb_x, w_x, psA)
        fps_x = emit_softmax(cps_x)

        # chain b PE work runs while softmax_x is on DVE/ACT
        emit_mirrors(g_sb_b, psB)
        cps_b = emit_Q_ctx(g_sb_b, w_b, psB)

        bd1 = emit_BD(fps_x, psA)                      # ctx1 (from x) -> out_b
        fps_b = emit_softmax(cps_b)

        # out_b: uses bd1 + xT_b; softmax_b hides under these matmuls
        for kb in range(NBIG):
            out_chunk(xT_b, bd1, kb, ob, psA)

        bd2 = emit_BD(fps_b, psB)                      # ctx2 (from blood) -> out_x
        for kb in range(NBIG):
            out_chunk(xT_x, bd2, kb, ox, psB)


_NC_CACHE = None


def _get_nc():
    global _NC_CACHE
    if _NC_CACHE is None:
        _NC_CACHE = build_nc()
    return _NC_CACHE


def _prep_inputs(x, blood, W1, W2):
    x = np.asarray(x, dtype=np.float32)
    blood = np.asarray(blood, dtype=np.float32)
    w1t = np.asarray(W1, dtype=np.float32).T.copy()
    w2t = np.asarray(W2, dtype=np.float32).T.copy()
    w1t[:, :C] *= SCALE  # fold softmax scale into the k-projection (exact: 2^-3)
    w2t[:, :C] *= SCALE
    x16 = x.astype(np.float16)
    b16 = blood.astype(np.float16)
    w1t16 = w1t.astype(np.float16)
    w2t16 = w2t.astype(np.float16)
    return [
        {
            "xb": np.ascontiguousarray(x16[b]),
            "bb": np.ascontiguousarray(b16[b]),
            "xbt": np.ascontiguousarray(x16[b].T),
            "bbt": np.ascontiguousarray(b16[b].T),
            "w1t": w1t16,
            "w2t": w2t16,
        }
        for b in range(B)
    ]


def _unshuffle(arr):
    """[NBIG, P, CB, 512] blocked-transposed fp16 -> [N, C] f32 natural."""
    # arr[kb, part, p, col] = out[kb*512 + col, p*128 + part]
    return np.ascontiguousarray(
        arr.transpose(0, 3, 2, 1).reshape(N, C).astype(np.float32))


def kernel(x, blood, W1, W2, trace=False):
    nc = _get_nc()
    in_maps = _prep_inputs(x, blood, W1, W2)
    res = run_bass_kernel_spmd(nc, in_maps, core_ids=list(range(B)), trace=trace)
    out_x = np.stack([_unshuffle(res.results[b]["oxT"]) for b in range(B)])
    out_b = np.stack([_unshuffle(res.results[b]["obT"]) for b in range(B)])
    if trace:
        kernel.last_results = res
    return (out_x, out_b)


# revision 10
# speedup vs baseline: 1.5564x; 1.1834x over previous
"""Trainium2 Bass kernel for nn_CrossAttention_31791347925417.

Math (per batch b, per stream tok in {x, blood} with weight W in {W1, W2}):
    kv = tok @ W.T ; k, v heads [H, N, D]
    ctx = softmax_d( SCALE * k_h^T v_h )          # [H, D, D], softmax over first D
    out_x = x_h @ ctx2_h ; out_b = blood_h @ ctx1_h

Gram trick: k_h^T v_h = W_k_h (tok^T tok) W_v_h^T with G = tok^T tok [C, C], so
the N=4096 contraction happens once per stream; everything downstream is tiny
[C,C]-scale work.  ctx probs are written into block-diagonal BD tiles used by
the output matmuls out[n, (h,e)] = sum_{(h,d)} tokT[(h,d), n] * BD[(h,d), (h,e)].

This version is fully fp16 on-chip (validated ~2e-3 rel err vs the 2e-2 gate):
  - Host supplies tokens BOTH natural [N, C] (for G) and pre-transposed [C, N]
    (for the output matmuls) in fp16, so there are no on-chip token transposes
    and no PSUM->SBUF cast traffic for them.
  - All matmuls run at 1 cycle/row (fp16); G uses the true upper triangle
    (G_OFF = [0,128,256,384]) and mirrors the 6 lower blocks via PE transposes.
  - Outputs are written fp16 and upcast on the host.

Sharding: data-parallel over batch B=8 across the 8 cores; weights replicated.
Host pre-transposes W -> W.T [C, 2C], folds SCALE into the k-half (exact 2^-3),
and casts to fp16.
"""

import sys

if "/opt/trn_rl_repo" not in sys.path:
    sys.path.insert(0, "/opt/trn_rl_repo")

import numpy as np

from concourse import bacc, masks, mybir, tile
from concourse.bass_utils import run_bass_kernel_spmd

B, N, C, H = 8, 4096, 512, 8
D = C // H
SCALE = D ** -0.5
P = 128
NBIG = N // 512          # 8 big row tiles (512 rows each)
NT = N // P              # 32 n-tiles
CB = C // P              # 4 column blocks == head pairs
F32 = mybir.dt.float32
F16 = mybir.dt.float16
AX = mybir.AxisListType
ACT_EXP = mybir.ActivationFunctionType.Exp

# G row-block m computes columns [G_OFF[m], C) -- true upper triangle.
G_OFF = [0, P, 2 * P, 3 * P]


def build_nc():
    nc = bacc.Bacc("TRN2", target_bir_lowering=False, debug=False)

    # host-packed SBUF-layout inputs: tokens [NBIG, P, 4C], xT [C, N],
    # weights [P, CB*2C] -- all fully contiguous per partition row.
    xb = nc.dram_tensor("xb", [NBIG, P, 4 * C], F16, kind="ExternalInput").ap()
    bb = nc.dram_tensor("bb", [NBIG, P, 4 * C], F16, kind="ExternalInput").ap()
    xbt = nc.dram_tensor("xbt", [C, N], F16, kind="ExternalInput").ap()
    bbt = nc.dram_tensor("bbt", [C, N], F16, kind="ExternalInput").ap()
    w1t = nc.dram_tensor("w1t", [P, CB * 2 * C], F16, kind="ExternalInput").ap()
    w2t = nc.dram_tensor("w2t", [P, CB * 2 * C], F16, kind="ExternalInput").ap()
    # blocked transposed output layout: [kb, part(c within pair), pair, n-col]
    ox = nc.dram_tensor("oxT", [NBIG, P, CB, 512], F16, kind="ExternalOutput").ap()
    ob = nc.dram_tensor("obT", [NBIG, P, CB, 512], F16, kind="ExternalOutput").ap()

    with tile.TileContext(nc) as tc:
        _emit(nc, tc, xb, bb, xbt, bbt, w1t, w2t, ox, ob)

    nc.compile()
    return nc


def _emit(nc, tc, xb, bb, xbt, bbt, w1t, w2t, ox, ob):
    from contextlib import ExitStack

    ctx = ExitStack()
    with ctx:
        const = ctx.enter_context(tc.tile_pool(name="const", bufs=1))
        wpool = ctx.enter_context(tc.tile_pool(name="wpool", bufs=1))
        tokp = ctx.enter_context(tc.tile_pool(name="tokp", bufs=16))
        xtp = ctx.enter_context(tc.tile_pool(name="xtp", bufs=1))
        gqp = ctx.enter_context(tc.tile_pool(name="gqp", bufs=16))
        smallp = ctx.enter_context(tc.tile_pool(name="smallp", bufs=2))
        fpool = ctx.enter_context(tc.tile_pool(name="fpool", bufs=2))
        bdpool = ctx.enter_context(tc.tile_pool(name="bdpool", bufs=8))
        ostp = ctx.enter_context(tc.tile_pool(name="ostp", bufs=4))
        psA = ctx.enter_context(tc.tile_pool(name="psA", bufs=4, space="PSUM"))
        psB = ctx.enter_context(tc.tile_pool(name="psB", bufs=4, space="PSUM"))

        ident = const.tile([P, P], F16, tag="idh")
        masks.make_identity(nc, ident[:])

        # weights: chunk j (c-rows 128j..128j+128) lives at cols [j*2C, (j+1)*2C)
        w_x = wpool.tile([P, CB * 2 * C], F16, tag="wx")
        w_b = wpool.tile([P, CB * 2 * C], F16, tag="wb")

        def load_weights():
            # w dram is host-packed [P, CB*2C]: contiguous per partition
            nc.sync.dma_start(w_x[:], w1t[:])
            nc.sync.dma_start(w_b[:], w2t[:])

        def wchunk(w, j):
            return w[:, j * 2 * C:(j + 1) * 2 * C]

        # transposed tokens (from host): pair block m at cols [m*N, (m+1)*N)
        xT_x = xtp.tile([P, CB * N], F16, tag="xtx")
        xT_b = xtp.tile([P, CB * N], F16, tag="xtb")

        def load_xT(xT, tdram):
            for m in range(CB):
                nc.sync.dma_start(
                    xT[:, m * N:(m + 1) * N], tdram[m * P:(m + 1) * P, :]
                )

        def emit_loads(tok_dram, pfx, split_first=False):
            # tok_dram is host-packed [NBIG, P, 4*C]: per big tile the full
            # 4KB partition row is contiguous in DRAM.
            toks = []
            for kb in range(NBIG):
                tokb = tokp.tile([P, 4 * C], F16, tag="tok", name=f"{pfx}{kb}")
                if kb == 0 and split_first:
                    for sub in range(4):
                        nc.sync.dma_start(
                            tokb[:, sub * C:(sub + 1) * C],
                            tok_dram[kb][:, sub * C:(sub + 1) * C],
                        )
                else:
                    nc.sync.dma_start(tokb[:], tok_dram[kb])
                toks.append(tokb)
            return toks

        def emit_G_tile(gps, sb, k):
            for m in range(CB):
                o = G_OFF[m]
                nc.tensor.matmul(
                    gps[m][:, o:C], sb[:, m * P:(m + 1) * P], sb[:, o:C],
                    start=(k == 0), stop=(k == NT - 1),
                )

        def drain_G(gps, eng):
            """PSUM G (f32) -> SBUF fp16 row-block tiles."""
            g_sb = []
            for m in range(CB):
                o = G_OFF[m]
                g = gqp.tile([P, C], F16, tag="gq", name=f"g{m}")
                eng(g[:, o:C], gps[m][:, o:C])
                g_sb.append(g)
            return g_sb

        def emit_mirrors(g_sb, pspool):
            """Fill missing lower blocks (i,j), j < G_OFF[i]//P, from (j,i)^T."""
            for i in range(CB):
                for j in range(G_OFF[i] // P):
                    mps = pspool.tile([P, P], F16, tag="g", name="mps")
                    nc.tensor.transpose(
                        mps[:], g_sb[j][:, i * P:(i + 1) * P], ident[:]
                    )
                    nc.vector.tensor_copy(g_sb[i][:, j * P:(j + 1) * P], mps[:])

        def emit_Q_ctx(g_sb, w, pspool):
            """Q = G @ Wk, then per-pair ctxT psum blocks (f32, kept in PSUM)."""
            q_sb = [None] * CB
            for i in reversed(range(CB)):
                qp = pspool.tile([P, C], F32, tag="g", name=f"qp{i}")
                for j in range(CB):
                    nc.tensor.matmul(
                        qp[:], g_sb[j][:, i * P:(i + 1) * P],
                        wchunk(w, j)[:, 0:C], start=(j == 0), stop=(j == 3),
                    )
                q = gqp.tile([P, C], F16, tag="gq", name=f"q{i}")
                nc.scalar.copy(q[:], qp[:])
                q_sb[i] = q
            cps_l = []
            for p in range(CB):
                cps = pspool.tile([P, P], F32, tag="g", name=f"cps{p}")
                for j in range(CB):
                    nc.tensor.matmul(
                        cps[:],
                        wchunk(w, j)[:, C + p * P:C + (p + 1) * P],
                        q_sb[j][:, p * P:(p + 1) * P],
                        start=(j == 0), stop=(j == 3),
                    )
                cps_l.append(cps)
            return cps_l

        def emit_softmax(cps_l):
            """softmax over d (free axis) of each diagonal 64x64 block; returns
            fp16 F tiles (block-diag probs, not yet transposed)."""
            fps = []
            for p in range(CB):
                cps = cps_l[p]
                nm = smallp.tile([P, 1], F32, tag="nm", name="nm")
                sm = smallp.tile([P, 1], F32, tag="sm", name="sm")
                rv = smallp.tile([P, 1], F32, tag="rv", name="rv")
                pp = smallp.tile([P, D], F32, tag="pp", name="pp")
                fp = fpool.tile([P, P], F16, tag="F", name="fp")
                nc.gpsimd.memset(fp[:], 0.0)
                for dd in range(2):
                    s0 = slice(dd * D, (dd + 1) * D)
                    blk = cps[s0, s0]
                    nc.vector.reduce_max(nm[s0, :], blk, axis=AX.X, negate=True)
                    nc.scalar.activation(
                        pp[s0, :], blk, ACT_EXP, bias=nm[s0, :], scale=1.0,
                        accum_out=sm[s0, :],
                    )
                nc.vector.reciprocal(rv[:], sm[:])
                for dd in range(2):
                    s0 = slice(dd * D, (dd + 1) * D)
                    nc.vector.tensor_scalar_mul(fp[s0, s0], pp[s0, :], rv[s0, :])
                fps.append(fp)
            return fps

        def emit_BD(fps, pspool):
            """PE-transpose prob tiles into block-diagonal ctx operands."""
            BDs = []
            for p in range(CB):
                bps = pspool.tile([P, P], F16, tag="g", name="bps")
                nc.tensor.transpose(bps[:], fps[p][:], ident[:])
                bd = bdpool.tile([P, P], F16, tag="bd", name=f"bd{p}")
                nc.vector.tensor_copy(bd[:], bps[:])
                BDs.append(bd)
            return BDs

        DRAIN = [nc.scalar.copy, nc.vector.tensor_copy]

        def out_chunk(xT, BDs, kb, odram, pool):
            """outT for 512 n-cols (tile-group kb): per pair p one matmul
            [c-block p, 512 n]; drain into ost quarter p rotating engines.
            `pool` alternates per chunk so 8 PSUM banks stay in flight."""
            ost = ostp.tile([P, 4 * 512], F16, tag="ost", name="ost")
            for p in range(CB):
                ops = pool.tile([P, 512], F32, tag="g", name=f"ops{p}")
                nc.tensor.matmul(
                    ops[:], BDs[p][:], xT[:, p * N + kb * 512:p * N + (kb + 1) * 512],
                    start=True, stop=True,
                )
                DRAIN[p % 2](ost[:, p * 512:(p + 1) * 512], ops[:])
            nc.scalar.dma_start(odram[kb], ost[:])

        # ---- emission schedule ----
        toks_x = emit_loads(xb, "tx", split_first=True)
        load_weights()
        toks_b = emit_loads(bb, "tb")
        load_xT(xT_b, bbt)
        load_xT(xT_x, xbt)

        # G_x: 32 k-tiles, 4 psum banks
        gps_x = [psA.tile([P, C], F32, tag="g", name=f"gpsx{m}") for m in range(CB)]
        for kb in range(NBIG):
            for sub in range(4):
                emit_G_tile(gps_x, toks_x[kb][:, sub * C:(sub + 1) * C], kb * 4 + sub)
        g_sb_x = drain_G(gps_x, nc.scalar.copy)       # scalar, runs during G_b

        # G_b first half
        gps_b = [psB.tile([P, C], F32, tag="g", name=f"gpsb{m}") for m in range(CB)]
        for kb in range(NBIG // 2):
            for sub in range(4):
                emit_G_tile(gps_b, toks_b[kb][:, sub * C:(sub + 1) * C], kb * 4 + sub)

        # mirrors for x slot in mid-G_b (their psum comes from freed psA banks)
        emit_mirrors(g_sb_x, psA)

        # G_b second half
        for kb in range(NBIG // 2, NBIG):
            for sub in range(4):
                emit_G_tile(gps_b, toks_b[kb][:, sub * C:(sub + 1) * C], kb * 4 + sub)
        g_sb_b = drain_G(gps_b, nc.vector.tensor_copy)  # DVE, frees scalar

        # chain x compute: Q, ctx (PE) then softmax (DVE/ACT)
        cps_x = emit_Q_ctx(g_sb_x, w_x, psA)
        fps_x = emit_softmax(cps_x)

        # chain b PE work runs while softmax_x is on DVE/ACT
        emit_mirrors(g_sb_b, psB)
        cps_b = emit_Q_ctx(g_sb_b, w_b, psB)

        bd1 = emit_BD(fps_x, psA)                      # ctx1 (from x) -> out_b
        fps_b = emit_softmax(cps_b)

        # out_b: uses bd1 + xT_b; softmax_b hides under these matmuls
        for kb in range(NBIG):
            out_chunk(xT_b, bd1, kb, ob, psA if kb % 2 == 0 else psB)

        bd2 = emit_BD(fps_b, psB)                      # ctx2 (from blood) -> out_x
        for kb in range(NBIG):
            out_chunk(xT_x, bd2, kb, ox, psB if kb % 2 == 0 else psA)


_NC_CACHE = None


def _get_nc():
    global _NC_CACHE
    if _NC_CACHE is None:
        _NC_CACHE = build_nc()
    return _NC_CACHE


def _prep_inputs(x, blood, W1, W2):
    x = np.asarray(x, dtype=np.float32)
    blood = np.asarray(blood, dtype=np.float32)
    w1t = np.asarray(W1, dtype=np.float32).T.copy()
    w2t = np.asarray(W2, dtype=np.float32).T.copy()
    w1t[:, :C] *= SCALE  # fold softmax scale into the k-projection (exact: 2^-3)
    w2t[:, :C] *= SCALE
    x16 = x.astype(np.float16)
    b16 = blood.astype(np.float16)

    def pack_tok(t):
        # [N, C] -> [NBIG, P, 4C] with tokb[kb][p, s*C + c] = t[kb*512+s*128+p, c]
        return np.ascontiguousarray(
            t.reshape(NBIG, 4, P, C).transpose(0, 2, 1, 3).reshape(NBIG, P, 4 * C))

    def pack_w(wt):
        # [C, 2C] -> [P, CB*2C] with w[p, j*2C + c] = wt[j*128+p, c]
        return np.ascontiguousarray(
            wt.reshape(CB, P, 2 * C).transpose(1, 0, 2).reshape(P, CB * 2 * C))

    w1p = pack_w(w1t.astype(np.float16))
    w2p = pack_w(w2t.astype(np.float16))
    return [
        {
            "xb": pack_tok(x16[b]),
            "bb": pack_tok(b16[b]),
            "xbt": np.ascontiguousarray(x16[b].T),
            "bbt": np.ascontiguousarray(b16[b].T),
            "w1t": w1p,
            "w2t": w2p,
        }
        for b in range(B)
    ]


def _unshuffle(arr):
    """[NBIG, P, CB, 512] blocked-transposed fp16 -> [N, C] f32 natural."""
    # arr[kb, part, p, col] = out[kb*512 + col, p*128 + part]
    return np.ascontiguousarray(
        arr.transpose(0, 3, 2, 1).reshape(N, C).astype(np.float32))


def kernel(x, blood, W1, W2, trace=False):
    nc = _get_nc()
    in_maps = _prep_inputs(x, blood, W1, W2)
    res = run_bass_kernel_spmd(nc, in_maps, core_ids=list(range(B)), trace=trace)
    out_x = np.stack([_unshuffle(res.results[b]["oxT"]) for b in range(B)])
    out_b = np.stack([_unshuffle(res.results[b]["obT"]) for b in range(B)])
    if trace:
        kernel.last_results = res
    return (out_x, out_b)
